# revision 42
# baseline (speedup 1.0000x reference)
"""DilateAttention Trainium2 Bass kernel.

Problem: per-pixel 3x3 dilated (dilation=2) local attention.
  q,k,v: [4, 192, 112, 112] f32 ; out: [4, 112, 112, 192] f32
  heads=6, head_dim=32, taps=9, zero-padded windows.

Strategy (8 NeuronCores, SPMD):
  * 24 (b,head) units -> channels flattened to 768; stacked in 6 groups of
    128 channels (4 units) on the partition axis.
  * Each group's 112 rows split into 4 quarters -> 24 chunks of
    [128 chan, 28 rows, 112 cols]; each core gets 3 chunks (all three
    prefetched into SBUF up-front, fp32->fp16 cast during DMA).
  * Per 2-row sub-block (224 pixels, 4 units):
      - DVE: one fused 9-tap x 2-row q (x) k product (fp16 2x mode,
        step-0 broadcast APs)
      - PE:  block-scale matmuls reduce 32-channel groups -> replicated
        per-unit logits in PSUM (scale 1/sqrt(32) folded into weights);
        attn split in two PSUM tiles so exp overlaps matmuls
      - ACT: exp(logits) -> SBUF fp16
      - DVE: one fused attn (x) v product
      - PE:  identity matmuls accumulate sum_k [prod | exp] -> PSUM
             giving unnormalized out and softmax denominator in one pass
  * Per sub-block PAIR: ACT 1/D via exp(-ln(D)), DVE final normalize
    multiply (fp32 out).
  * k/v zero-padding done host-side (np.pad) which reproduces the
    reference's softmax-over-zero-logits boundary semantics exactly.
"""

import numpy as np
from contextlib import ExitStack

import concourse.bass as bass
import concourse.tile as tile
from concourse import mybir
from concourse.bass_utils import run_bass_kernel_spmd

# ---------------------------------------------------------------- constants
B, C, H, W = 4, 192, 112, 112
NUM_HEADS, HEAD_DIM = 6, 32
KK = 9
PAD = 2
HP, WP = H + 2 * PAD, W + 2 * PAD  # 116, 116
G = B * C  # 768 flattened channels
N_GROUPS = 6  # 768 / 128
N_CORES = 8
CHUNK_ROWS = 28  # rows per chunk (112 / 4)
N_CHUNKS_PER_CORE = 3  # 24 chunks / 8 cores
SCALE = HEAD_DIM ** -0.5

F16 = mybir.dt.float16
F32 = mybir.dt.float32


def _view(ap, extra_offset, dims):
    """Free-dim access-pattern view on an SBUF/PSUM tile AP.

    `dims` = list of [step, count] in elements (innermost last); partition
    dim is taken from the base AP.
    """
    base = ap[:] if not isinstance(ap, bass.AP) else ap
    part = base.ap[0]
    return bass.AP(
        tensor=base.tensor,
        offset=base.offset + extra_offset,
        ap=[part] + [list(d) for d in dims],
    )


def build_nc(reps=1, chunk_rows=CHUNK_ROWS, n_chunks=N_CHUNKS_PER_CORE,
             skip=frozenset(), stt_products=False, dve_norm=True,
             pipeline=True, norm_engine="pool_div", od_bufs=2, ct_bufs=4,
             att_bufs=2, prod_bufs=3, egress_dve=False, recip_act=False):
    """Build the per-core Bass program (SPMD: same program, per-core data)."""
    kr = chunk_rows + 4  # padded k/v rows per chunk
    n_sub = chunk_rows // 2  # 2-row sub-blocks per chunk
    assert n_sub % 2 == 0 or n_sub == 1

    nc = bass.Bass("TRN2", target_bir_lowering=False, debug=False,
                   num_devices=N_CORES)

    q_d = nc.dram_tensor("q", [n_chunks, 128, chunk_rows, W], F32,
                         kind="ExternalInput").ap()
    k_d = nc.dram_tensor("k", [n_chunks, 128, kr, WP], F32,
                         kind="ExternalInput").ap()
    v_d = nc.dram_tensor("v", [n_chunks, 128, kr, WP], F32,
                         kind="ExternalInput").ap()
    bs_d = nc.dram_tensor("bs", [128, 128], F16, kind="ExternalInput").ap()
    id_d = nc.dram_tensor("id128", [128, 128], F16, kind="ExternalInput").ap()
    o_d = nc.dram_tensor("o", [n_chunks, 128, chunk_rows, W], F32,
                         kind="ExternalOutput").ap()

    with tile.TileContext(nc) as tc:
        with ExitStack() as ctx:
            consts = ctx.enter_context(tc.tile_pool(name="consts", bufs=1))
            qpool = ctx.enter_context(
                tc.tile_pool(name="qpool", bufs=n_chunks))
            kpool = ctx.enter_context(
                tc.tile_pool(name="kpool", bufs=n_chunks))
            vpool = ctx.enter_context(
                tc.tile_pool(name="vpool", bufs=n_chunks))
            opool = ctx.enter_context(tc.tile_pool(name="opool", bufs=2))
            prodp = ctx.enter_context(
                tc.tile_pool(name="prodp", bufs=prod_bufs if pipeline else 4))
            cp = ctx.enter_context(
                tc.tile_pool(name="cp", bufs=ct_bufs if pipeline else 6))
            drp = ctx.enter_context(tc.tile_pool(name="drp", bufs=6))
            att_ps = ctx.enter_context(
                tc.tile_pool(name="att_ps", bufs=att_bufs if pipeline else 1,
                             space="PSUM"))
            od_ps = ctx.enter_context(
                tc.tile_pool(name="od_ps", bufs=od_bufs if pipeline else 3,
                             space="PSUM"))

            bs_sb = consts.tile([128, 128], F16)
            nc.sync.dma_start(out=bs_sb[:], in_=bs_d[:])
            id_sb = consts.tile([128, 128], F16)
            nc.sync.dma_start(out=id_sb[:], in_=id_d[:])
            ones_sb = consts.tile([128, 1], F32)
            nc.vector.memset(ones_sb[:], 1.0)

            def prefetch(defer=False):
                """SWDGE triggers serialize on the Pool engine (~1us each),
                which also runs the normalize. Chunk 0 streams in two
                consumption-ordered row groups up front; later chunks'
                triggers are deferred into stageF slots (after each norm)
                when `defer` is set, so they soak Pool slack instead of
                delaying the first norms."""
                qs, ks, vs = [], [], []
                deferred = []
                for t in range(n_chunks):
                    q_sb = qpool.tile([128, chunk_rows, W], F16, tag="q")
                    k_sb = kpool.tile([128, kr, WP], F16, tag="k")
                    v_sb = vpool.tile([128, kr, WP], F16, tag="v")
                    if t == 0 and chunk_rows > 16 and pipeline:
                        # q first: the first QK product needs q+k rows 0-1;
                        # v is only needed one slot later
                        nc.gpsimd.dma_start(out=q_sb[:, 0:8, :],
                                            in_=q_d[t, :, 0:8, :])
                        nc.gpsimd.dma_start(out=k_sb[:, 0:8, :],
                                            in_=k_d[t, :, 0:8, :])
                        nc.gpsimd.dma_start(out=v_sb[:, 0:8, :],
                                            in_=v_d[t, :, 0:8, :])
                        nc.gpsimd.dma_start(out=k_sb[:, 8:, :],
                                            in_=k_d[t, :, 8:, :])
                        nc.gpsimd.dma_start(out=v_sb[:, 8:, :],
                                            in_=v_d[t, :, 8:, :])
                        nc.gpsimd.dma_start(out=q_sb[:, 8:, :],
                                            in_=q_d[t, :, 8:, :])
                    elif defer and t > 0:
                        deferred.append((q_sb, k_sb, v_sb, t))
                    elif t == 0 and chunk_rows > 16:
                        nc.gpsimd.dma_start(out=q_sb[:, 0:8, :],
                                            in_=q_d[t, :, 0:8, :])
                        nc.gpsimd.dma_start(out=k_sb[:, 0:12, :],
                                            in_=k_d[t, :, 0:12, :])
                        nc.gpsimd.dma_start(out=v_sb[:, 0:12, :],
                                            in_=v_d[t, :, 0:12, :])
                        nc.gpsimd.dma_start(out=q_sb[:, 8:, :],
                                            in_=q_d[t, :, 8:, :])
                        nc.gpsimd.dma_start(out=k_sb[:, 12:, :],
                                            in_=k_d[t, :, 12:, :])
                        nc.gpsimd.dma_start(out=v_sb[:, 12:, :],
                                            in_=v_d[t, :, 12:, :])
                    else:
                        nc.gpsimd.dma_start(out=q_sb[:], in_=q_d[t])
                        nc.gpsimd.dma_start(out=k_sb[:], in_=k_d[t])
                        nc.gpsimd.dma_start(out=v_sb[:], in_=v_d[t])
                    qs.append(q_sb), ks.append(k_sb), vs.append(v_sb)
                return qs, ks, vs, deferred

            def body_pipelined():
                """Software-pipelined emission: per slot i the engines see
                  DVE:  QK-TT(i) r0,r1   then AV-TT(i-1) r0,r1
                  PE :  QK-mm(i) x6      then OD-mm(i-1) x9
                  ACT:  exp(i) r0,r1
                  Pool: 1/D(i-2), out-mult(i-2)
                so every engine has ready work each slot. att is double-
                buffered per ROW ([128,9,128] f32, bank-aligned 512B/tap)
                which fits 2 bufs + od 2 bufs in the 16KB PSUM budget."""
                qs, ks, vs, deferred = prefetch(defer=True)
                # one deferred prefetch trigger per early stageF slot
                trigs = []
                for q_sb, k_sb, v_sb, t in deferred:
                    trigs.append((k_sb, k_d[t]))
                    trigs.append((v_sb, v_d[t]))
                    trigs.append((q_sb, q_d[t]))
                total = n_chunks * n_sub
                prods, cts, atts, ods, osbs = {}, {}, {}, {}, {}

                def stageA(i):  # DVE: QK products
                    t, sb = divmod(i, n_sub)
                    rb = 2 * sb
                    prod = prodp.tile([128, 2, KK, W], F16, tag="prod")
                    prods[i] = prod
                    for rl in range(2):
                        kv = _view(ks[t], (rb + rl) * WP,
                                   [[2 * WP, 3], [2, 3], [1, W]])
                        qv = _view(qs[t], (rb + rl) * W,
                                   [[0, 3], [0, 3], [1, W]])
                        pv = _view(prod, rl * KK * W,
                                   [[3 * W, 3], [W, 3], [1, W]])
                        nc.vector.tensor_tensor(
                            pv, kv, qv, mybir.AluOpType.mult)

                def stageB(i):  # PE: logits matmuls (per row, 4+4+1 taps)
                    # per-row att [128, 9, 128] f32: 512B/tap slot so every
                    # 448B matmul write stays inside a 2KB PSUM bank; row
                    # tiles ping-pong through the pool's 2 bufs
                    prod = prods[i]
                    row_atts = []
                    for rl in range(2):
                        att = att_ps.tile([128, KK, 128], F32, tag="att")
                        row_atts.append(att)
                        for g, ng in ((0, 4), (4, 4), (8, 1)):
                            rhs = _view(prod, rl * KK * W + g * W,
                                        [[W, ng], [1, W]])
                            out = _view(att, g * 128, [[128, ng], [1, W]])
                            nc.tensor.matmul(out, bs_sb[:], rhs,
                                             start=True, stop=True)
                    atts[i] = row_atts

                def stageC(i):  # ACT: exp per row -> c_t exp half
                    c_t = cp.tile([128, KK, 2, 224], F16, tag="C")
                    cts[i] = c_t
                    for rl in range(2):
                        att = atts[i][rl]
                        attn_v = _view(att, 0, [[128, KK], [1, W]])
                        exp_out = _view(c_t, 224 + rl * W,
                                        [[448, KK], [1, W]])
                        nc.scalar.activation(
                            exp_out, attn_v,
                            mybir.ActivationFunctionType.Exp)
                    del atts[i]

                def stageD(i):  # DVE: AV products
                    t, sb = divmod(i, n_sub)
                    rb = 2 * sb
                    c_t = cts[i]
                    for rl in range(2):
                        vv = _view(vs[t], (rb + rl) * WP,
                                   [[2 * WP, 3], [2, 3], [1, W]])
                        ev = _view(c_t, 224 + rl * W,
                                   [[3 * 448, 3], [448, 3], [1, W]])
                        p2 = _view(c_t, rl * W,
                                   [[3 * 448, 3], [448, 3], [1, W]])
                        nc.vector.tensor_tensor(
                            p2, vv, ev, mybir.AluOpType.mult)

                def stageE(i):  # PE: accumulate sum_k [prod | exp]
                    c_t = cts[i]
                    od = od_ps.tile([128, 448], F32, tag="od")
                    for kk in range(KK):
                        rhs = _view(c_t, kk * 448, [[1, 448]])
                        nc.tensor.matmul(od[:], id_sb[:], rhs,
                                         start=(kk == 0), stop=(kk == KK - 1))
                    del cts[i]
                    ods[i] = od

                def stageF(i):  # Pool: dr = 1/D ; o = Oun*dr ; out DMA
                    t, sb = divmod(i, n_sub)
                    rb = 2 * sb
                    if sb == 0:
                        osbs[t] = opool.tile([128, chunk_rows, W], F32,
                                             tag="o", name="o_sb")
                    o_sb = osbs[t]
                    od = ods.pop(i)
                    dr = drp.tile([128, 224], F32, tag="dr")
                    out_v = _view(o_sb, rb * W, [[1, 224]])
                    if norm_engine == "pool_div":
                        # Division exists on no engine; GPSIMD cannot touch
                        # PSUM. Spread the normalize across the three
                        # non-critical paths: DVE takes 1/D (only DVE has
                        # reciprocal), ACT copies the AV half out of PSUM,
                        # Pool does the final elementwise multiply in SBUF.
                        av_sb = drp.tile([128, 224], F32, tag="av_sb")
                        nc.scalar.copy(av_sb[:], od[:, 0:224])
                        if i % 4 == 3 and recip_act:
                            lnd = drp.tile([128, 224], F32, tag="lnd")
                            nc.scalar.activation(
                                lnd[:], od[:, 224:448],
                                mybir.ActivationFunctionType.Ln)
                            nc.scalar.activation(
                                dr[:], lnd[:],
                                mybir.ActivationFunctionType.Exp,
                                scale=-1.0)
                        else:
                            nc.vector.reciprocal(dr[:], od[:, 224:448])
                        nc.gpsimd.tensor_tensor(
                            out_v, av_sb[:], dr[:],
                            mybir.AluOpType.mult)
                    elif norm_engine == "dve":
                        nc.vector.reciprocal(dr[:], od[:, 224:448])
                        nc.vector.tensor_tensor(
                            out_v, od[:, 0:224], dr[:],
                            mybir.AluOpType.mult)
                    else:  # act
                        lnd = drp.tile([128, 224], F32, tag="lnd")
                        nc.scalar.activation(
                            lnd[:], od[:, 224:448],
                            mybir.ActivationFunctionType.Ln)
                        nc.scalar.activation(
                            dr[:], lnd[:],
                            mybir.ActivationFunctionType.Exp, scale=-1.0)
                        nc.vector.tensor_tensor(
                            out_v, od[:, 0:224], dr[:],
                            mybir.AluOpType.mult)
                    # output DMA: half at mid-chunk, then quarters so the
                    # tail transfer after the very last norm is small
                    if n_sub >= 8:
                        # DMA boundaries must sit on 2-row sub-block edges
                        h = 2 * (n_sub // 2)        # rows done by sb h/2-1
                        q3 = 2 * (3 * n_sub // 4)   # rows done by sb 3n/4-1
                        last = t == n_chunks - 1
                        if sb == n_sub // 2 - 1:
                            nc.sync.dma_start(out=o_d[t, :, 0:h, :],
                                              in_=o_sb[:, 0:h, :])
                        elif sb == 3 * n_sub // 4 - 1:
                            nc.sync.dma_start(out=o_d[t, :, h:q3, :],
                                              in_=o_sb[:, h:q3, :])
                        elif last and sb == n_sub - 2:
                            nc.sync.dma_start(
                                out=o_d[t, :, q3:chunk_rows - 2, :],
                                in_=o_sb[:, q3:chunk_rows - 2, :])
                        elif sb == n_sub - 1:
                            r0 = chunk_rows - 2 if last else q3
                            nc.sync.dma_start(out=o_d[t, :, r0:, :],
                                              in_=o_sb[:, r0:, :])
                    elif sb == n_sub - 1:
                        nc.sync.dma_start(out=o_d[t], in_=o_sb[:])

                for i in range(total + 2):
                    if i < total:
                        stageA(i)
                        stageB(i)
                        stageC(i)
                    if 0 <= i - 1 < total:
                        stageD(i - 1)
                        stageE(i - 1)
                    if 0 <= i - 2 < total:
                        stageF(i - 2)
                        if i - 2 >= 4 and (i - 2) % 2 == 0 and trigs:
                            sb_out, d_in = trigs.pop(0)
                            nc.gpsimd.dma_start(out=sb_out[:], in_=d_in)

            def body():
                # prefetch all chunks (cast f32 -> f16 during DMA).
                # Chunk 0 is loaded in two slices so the first sub-blocks
                # can start while the rest streams in.
                qs, ks, vs, _ = prefetch()

                for t in range(n_chunks):
                    q_sb, k_sb, v_sb = qs[t], ks[t], vs[t]
                    o_sb = opool.tile([128, chunk_rows, W], F32, tag="o")

                    for sb in range(n_sub):
                        rb = 2 * sb  # first out-row of sub-block
                        od = od_ps.tile([128, 448], F32, tag="od")
                        if True:
                            # prod: [128, 2(row), 9(kk), 112] fp16
                            prod = prodp.tile([128, 2, KK, W], F16,
                                              tag="prod")
                            # C: [128, 9(kk), 2(half), 224];
                            # half0=AV prod, half1=exp
                            c_t = cp.tile([128, KK, 2, 224], F16, tag="C")
                            attn_a = att_ps.tile([128, 5, 256], F32,
                                                 tag="attnA")
                            attn_b = att_ps.tile([128, 4, 256], F32,
                                                 tag="attnB")

                            # ---- QK products (per row, fused over 9 taps;
                            # walrus caps TT APs at 3 free dims).
                            # scalar_tensor_tensor (out = (in0*1)*in1) hits
                            # the DVE 4x perf mode; tensor_tensor caps at 2x.
                            if "qk_tt" not in skip:
                                for rl in range(2):
                                    kv = _view(k_sb, (rb + rl) * WP,
                                               [[2 * WP, 3], [2, 3], [1, W]])
                                    qv = _view(q_sb, (rb + rl) * W,
                                               [[0, 3], [0, 3], [1, W]])
                                    pv = _view(prod, rl * KK * W,
                                               [[3 * W, 3], [W, 3], [1, W]])
                                    if stt_products:
                                        nc.vector.scalar_tensor_tensor(
                                            pv, kv, 1.0, qv,
                                            mybir.AluOpType.mult,
                                            mybir.AluOpType.mult)
                                    else:
                                        nc.vector.tensor_tensor(
                                            pv, kv, qv, mybir.AluOpType.mult)

                            # ---- logits matmuls + exp per attn half
                            for att_t, k0, nk in ((attn_a, 0, 5),
                                                  (attn_b, 5, 4)):
                                if "qk_mm" not in skip:
                                    for kl in range(nk):
                                        kk = k0 + kl
                                        rhs = _view(prod, kk * W,
                                                    [[KK * W, 2], [1, W]])
                                        nc.tensor.matmul(
                                            att_t[:, kl, 0:224], bs_sb[:],
                                            rhs, start=True, stop=True)
                                if "exp" not in skip:
                                    attn_v = _view(att_t, 0,
                                                   [[256, nk], [1, 224]])
                                    exp_out = _view(c_t, k0 * 448 + 224,
                                                    [[448, nk], [1, 224]])
                                    nc.scalar.activation(
                                        exp_out, attn_v,
                                        mybir.ActivationFunctionType.Exp)

                            # ---- AV products (per row, fused over 9 taps)
                            if "av_tt" not in skip:
                                for rl in range(2):
                                    vv = _view(v_sb, (rb + rl) * WP,
                                               [[2 * WP, 3], [2, 3], [1, W]])
                                    ev = _view(c_t, 224 + rl * W,
                                               [[3 * 448, 3], [448, 3],
                                                [1, W]])
                                    p2 = _view(c_t, rl * W,
                                               [[3 * 448, 3], [448, 3],
                                                [1, W]])
                                    if stt_products:
                                        nc.vector.scalar_tensor_tensor(
                                            p2, vv, 1.0, ev,
                                            mybir.AluOpType.mult,
                                            mybir.AluOpType.mult)
                                    else:
                                        nc.vector.tensor_tensor(
                                            p2, vv, ev, mybir.AluOpType.mult)

                            # ---- accumulate sum_k [prod | exp] on PE
                            if "od_mm" not in skip:
                                for kk in range(KK):
                                    rhs = _view(c_t, kk * 448, [[1, 448]])
                                    nc.tensor.matmul(
                                        od[:], id_sb[:], rhs,
                                        start=(kk == 0), stop=(kk == KK - 1))

                        # ---- normalize: dr = 1/D; o = Oun*dr
                        if "norm" not in skip:
                            dr = drp.tile([128, 224], F32, tag="dr")
                            if dve_norm:
                                nc.vector.reciprocal(dr[:], od[:, 224:448])
                            else:
                                lnd = drp.tile([128, 224], F32, tag="lnd")
                                nc.scalar.activation(
                                    lnd[:], od[:, 224:448],
                                    mybir.ActivationFunctionType.Ln)
                                nc.scalar.activation(
                                    dr[:], lnd[:],
                                    mybir.ActivationFunctionType.Exp,
                                    scale=-1.0)
                            out_v = _view(o_sb, rb * W, [[1, 224]])
                            nc.vector.tensor_tensor(
                                out_v, od[:, 0:224], dr[:],
                                mybir.AluOpType.mult)

                        if n_sub >= 8 and sb == n_sub // 2 - 1:
                            nc.sync.dma_start(
                                out=o_d[t, :, 0:chunk_rows // 2, :],
                                in_=o_sb[:, 0:chunk_rows // 2, :])

                    if n_sub >= 8:
                        # second half only; first half was sent mid-chunk
                        nc.sync.dma_start(
                            out=o_d[t, :, chunk_rows // 2:, :],
                            in_=o_sb[:, chunk_rows // 2:, :])
                    else:
                        nc.sync.dma_start(out=o_d[t], in_=o_sb[:])

            # NB: tc.For_i emits raw-ISA register/branch ops this container's
            # walrus rejects ("ISA wrong length") -> python-unroll reps.
            for _ in range(reps):
                if pipeline:
                    body_pipelined()
                else:
                    body()

    return nc


def _split_waits(nc, max_waits=1):
    """walrus in this container rejects >1 sync-wait per instruction;
    move extra waits onto preceding NOPs."""
    for fn in nc.m.functions:
        for blk in fn.blocks:
            insts = blk.instructions
            new_insts = []
            for inst in insts:
                si = getattr(inst, "sync_info", None)
                if si is not None and si.on_wait and len(si.on_wait) > max_waits:
                    waits = list(si.on_wait)
                    extra, keep = waits[:-max_waits], waits[-max_waits:]
                    k = 0
                    while extra:
                        chunk, extra = extra[:max_waits], extra[max_waits:]
                        new_insts.append(mybir.InstNoOp(
                            name=f"{inst.name}-ws{k}",
                            engine=inst.engine,
                            sync_info=mybir.SyncInfo(on_wait=chunk,
                                                     on_update=[]),
                            bass_nofuse=True,
                        ))
                        k += 1
                    inst.sync_info = mybir.SyncInfo(
                        on_wait=keep, on_update=list(si.on_update))
                new_insts.append(inst)
            blk.instructions.clear()
            blk.instructions.extend(new_insts)


# ------------------------------------------------------------- host helpers
def make_consts():
    bs = np.zeros((128, 128), np.float16)
    for u in range(4):
        bs[u * 32:(u + 1) * 32, u * 32:(u + 1) * 32] = np.float16(SCALE)
    return bs, np.eye(128, dtype=np.float16)


def shard_inputs(q, k, v):
    """Full [4,192,112,112] f32 -> per-core input maps."""
    qf = np.ascontiguousarray(q.reshape(G, H, W))
    kp = np.pad(k, ((0, 0), (0, 0), (PAD, PAD), (PAD, PAD))).reshape(G, HP, WP)
    vp = np.pad(v, ((0, 0), (0, 0), (PAD, PAD), (PAD, PAD))).reshape(G, HP, WP)
    bs, id128 = make_consts()
    in_maps = []
    for c in range(N_CORES):
        qs, ks, vs = [], [], []
        for t in range(N_CHUNKS_PER_CORE):
            ci = c * N_CHUNKS_PER_CORE + t
            g, qt = divmod(ci, 4)
            r0 = CHUNK_ROWS * qt
            sl = slice(128 * g, 128 * (g + 1))
            qs.append(qf[sl, r0:r0 + CHUNK_ROWS, :])
            ks.append(kp[sl, r0:r0 + CHUNK_ROWS + 4, :])
            vs.append(vp[sl, r0:r0 + CHUNK_ROWS + 4, :])
        in_maps.append({
            "q": np.ascontiguousarray(np.stack(qs)),
            "k": np.ascontiguousarray(np.stack(ks)),
            "v": np.ascontiguousarray(np.stack(vs)),
            "bs": bs,
            "id128": id128,
        })
    return in_maps


def assemble_output(results):
    """Per-core 'o' [3,128,28,112] f32 -> full [4,112,112,192]."""
    oc = np.empty((G, H, W), np.float32)
    for c in range(N_CORES):
        for t in range(N_CHUNKS_PER_CORE):
            ci = c * N_CHUNKS_PER_CORE + t
            g, qt = divmod(ci, 4)
            r0 = CHUNK_ROWS * qt
            oc[128 * g:128 * (g + 1), r0:r0 + CHUNK_ROWS, :] = \
                results[c]["o"][t]
    return np.ascontiguousarray(
        oc.reshape(B, C, H, W).transpose(0, 2, 3, 1))


_NC_CACHE = {}


def kernel(q, k, v):
    key = "main"
    if key not in _NC_CACHE:
        nc_new = build_nc()
        _split_waits(nc_new)
        _NC_CACHE[key] = nc_new
    nc = _NC_CACHE[key]
    in_maps = shard_inputs(np.asarray(q), np.asarray(k), np.asarray(v))
    res = run_bass_kernel_spmd(nc, in_maps, list(range(N_CORES)))
    return assemble_output(res.results)



# revision 53
# speedup vs baseline: 15.4269x; 15.4269x over previous
"""DilateAttention Trainium2 Bass kernel.

Problem: per-pixel 3x3 dilated (dilation=2) local attention.
  q,k,v: [4, 192, 112, 112] f32 ; out: [4, 112, 112, 192] f32
  heads=6, head_dim=32, taps=9, zero-padded windows.

Strategy (8 NeuronCores, SPMD):
  * 24 (b,head) units -> channels flattened to 768; stacked in 6 groups of
    128 channels (4 units) on the partition axis.
  * Each group's 112 rows split into 4 quarters -> 24 chunks of
    [128 chan, 28 rows, 112 cols]; each core gets 3 chunks (all three
    prefetched into SBUF up-front, fp32->fp16 cast during SWDGE DMA,
    chunk 0 in consumption-ordered slices, chunks 1-2 deferred into
    early normalize slots so Pool trigger bursts don't delay norms).
  * SOFTWARE-PIPELINED sub-block stream (2-row sub-blocks; per slot i
    every engine has ready work):
      - DVE:  QK-TT(i) r0,r1 (fused 9-tap q*k products, fp16 2x mode)
              then AV-TT(i-1) r0,r1, then 1/D reciprocal(i-2)
      - PE :  QK matmuls(i) per row in 4+4+1-tap groups into per-row
              PSUM att tiles [128,9,128] (512B/tap keeps each 448B
              write inside a 2KB bank; 2-buf ping-pong), then OD
              identity-matmuls(i-1) accumulating sum_k [prod | exp]
              (unnormalized out + softmax denominator in one pass)
      - ACT:  exp(i) per row -> SBUF fp16, then od(i-2) PSUM->SBUF copy
              (GPSIMD can't read PSUM, DMA can't either)
      - Pool: final normalize multiply out = od_av * (1/D) in SBUF
  * Output DMA per half/quarter chunk; the very last transfer is 2 rows
    so the drain tail stays short.
  * k/v zero-padding done host-side (np.pad) which reproduces the
    reference's softmax-over-zero-logits boundary semantics exactly.
Engine budget per 2-row sub-block (42/core, cost-model ns):
  PE 2520 | DVE 2698 | ACT 2422 | Pool ~950  -> ~130us/core simulated
  (baseline before pipelining: 149.7us sim / 144.8us measured).
"""

import numpy as np
from contextlib import ExitStack

import concourse.bass as bass
import concourse.tile as tile
from concourse import mybir
from concourse.bass_utils import run_bass_kernel_spmd

# ---------------------------------------------------------------- constants
B, C, H, W = 4, 192, 112, 112
NUM_HEADS, HEAD_DIM = 6, 32
KK = 9
PAD = 2
HP, WP = H + 2 * PAD, W + 2 * PAD  # 116, 116
G = B * C  # 768 flattened channels
N_GROUPS = 6  # 768 / 128
N_CORES = 8
CHUNK_ROWS = 28  # rows per chunk (112 / 4)
N_CHUNKS_PER_CORE = 3  # 24 chunks / 8 cores
SCALE = HEAD_DIM ** -0.5

F16 = mybir.dt.float16
F32 = mybir.dt.float32


def _view(ap, extra_offset, dims):
    """Free-dim access-pattern view on an SBUF/PSUM tile AP.

    `dims` = list of [step, count] in elements (innermost last); partition
    dim is taken from the base AP.
    """
    base = ap[:] if not isinstance(ap, bass.AP) else ap
    part = base.ap[0]
    return bass.AP(
        tensor=base.tensor,
        offset=base.offset + extra_offset,
        ap=[part] + [list(d) for d in dims],
    )


def build_nc(reps=1, chunk_rows=CHUNK_ROWS, n_chunks=N_CHUNKS_PER_CORE,
             skip=frozenset(), stt_products=False, dve_norm=True,
             pipeline=True, norm_engine="pool_div", od_bufs=2, ct_bufs=5,
             att_bufs=2, prod_bufs=2, egress_dve=False, recip_act=False,
             head32=False):
    """Build the per-core Bass program (SPMD: same program, per-core data)."""
    kr = chunk_rows + 4  # padded k/v rows per chunk
    n_sub = chunk_rows // 2  # 2-row sub-blocks per chunk
    assert n_sub % 2 == 0 or n_sub == 1

    nc = bass.Bass("TRN2", target_bir_lowering=False, debug=False,
                   num_devices=N_CORES)

    q_d = nc.dram_tensor("q", [n_chunks, 128, chunk_rows, W], F32,
                         kind="ExternalInput").ap()
    k_d = nc.dram_tensor("k", [n_chunks, 128, kr, WP], F32,
                         kind="ExternalInput").ap()
    v_d = nc.dram_tensor("v", [n_chunks, 128, kr, WP], F32,
                         kind="ExternalInput").ap()
    bs_d = nc.dram_tensor("bs", [128, 128], F16, kind="ExternalInput").ap()
    id_d = nc.dram_tensor("id128", [128, 128], F16, kind="ExternalInput").ap()
    o_d = nc.dram_tensor("o", [n_chunks, 128, chunk_rows, W], F32,
                         kind="ExternalOutput").ap()

    with tile.TileContext(nc) as tc:
        with ExitStack() as ctx:
            consts = ctx.enter_context(tc.tile_pool(name="consts", bufs=1))
            qpool = ctx.enter_context(
                tc.tile_pool(name="qpool", bufs=n_chunks))
            kpool = ctx.enter_context(
                tc.tile_pool(name="kpool", bufs=n_chunks))
            vpool = ctx.enter_context(
                tc.tile_pool(name="vpool", bufs=n_chunks))
            opool = ctx.enter_context(tc.tile_pool(name="opool", bufs=2))
            prodp = ctx.enter_context(
                tc.tile_pool(name="prodp", bufs=prod_bufs if pipeline else 4))
            cp = ctx.enter_context(
                tc.tile_pool(name="cp", bufs=ct_bufs if pipeline else 6))
            drp = ctx.enter_context(tc.tile_pool(name="drp", bufs=6))
            att_ps = ctx.enter_context(
                tc.tile_pool(name="att_ps", bufs=att_bufs if pipeline else 1,
                             space="PSUM"))
            od_ps = ctx.enter_context(
                tc.tile_pool(name="od_ps", bufs=od_bufs if pipeline else 3,
                             space="PSUM"))

            bs_sb = consts.tile([128, 128], F16)
            nc.sync.dma_start(out=bs_sb[:], in_=bs_d[:])
            id_sb = consts.tile([128, 128], F16)
            nc.sync.dma_start(out=id_sb[:], in_=id_d[:])
            ones_sb = consts.tile([128, 1], F32)
            nc.vector.memset(ones_sb[:], 1.0)

            def prefetch(defer=False):
                """SWDGE triggers serialize on the Pool engine (~1us each),
                which also runs the normalize. Chunk 0 streams in two
                consumption-ordered row groups up front; later chunks'
                triggers are deferred into stageF slots (after each norm)
                when `defer` is set, so they soak Pool slack instead of
                delaying the first norms."""
                qs, ks, vs = [], [], []
                deferred = []
                heads = {}
                for t in range(n_chunks):
                    q_sb = qpool.tile([128, chunk_rows, W], F16, tag="q")
                    k_sb = kpool.tile([128, kr, WP], F16, tag="k")
                    v_sb = vpool.tile([128, kr, WP], F16, tag="v")
                    if t == 0 and chunk_rows > 16 and pipeline:
                        # q first: the first QK product needs q+k rows 0-1;
                        # v is only needed one slot later
                        nc.gpsimd.dma_start(out=q_sb[:, 0:8, :],
                                            in_=q_d[t, :, 0:8, :])
                        nc.gpsimd.dma_start(out=k_sb[:, 0:8, :],
                                            in_=k_d[t, :, 0:8, :])
                        nc.gpsimd.dma_start(out=v_sb[:, 0:8, :],
                                            in_=v_d[t, :, 0:8, :])
                        nc.gpsimd.dma_start(out=k_sb[:, 8:, :],
                                            in_=k_d[t, :, 8:, :])
                        nc.gpsimd.dma_start(out=v_sb[:, 8:, :],
                                            in_=v_d[t, :, 8:, :])
                        nc.gpsimd.dma_start(out=q_sb[:, 8:, :],
                                            in_=q_d[t, :, 8:, :])
                        if head32:
                            # f32 head tiles via the parallel HWDGE queues
                            # feed sub-block 0 ~3us before the Pool SWDGE
                            # stream lands (its products run 1x f32, one-off)
                            qh = qpool.tile([128, 2, W], F32, tag="qh",
                                            name="q_head32")
                            kh = kpool.tile([128, 6, WP], F32, tag="kh",
                                            name="k_head32")
                            vh = vpool.tile([128, 6, WP], F32, tag="vh",
                                            name="v_head32")
                            nc.sync.dma_start(out=kh[:],
                                              in_=k_d[t, :, 0:6, :])
                            nc.scalar.dma_start(out=qh[:],
                                                in_=q_d[t, :, 0:2, :])
                            nc.sync.dma_start(out=vh[:],
                                              in_=v_d[t, :, 0:6, :])
                            heads.update(qh=qh, kh=kh, vh=vh)
                    elif defer and t > 0:
                        deferred.append((q_sb, k_sb, v_sb, t))
                    elif t == 0 and chunk_rows > 16:
                        nc.gpsimd.dma_start(out=q_sb[:, 0:8, :],
                                            in_=q_d[t, :, 0:8, :])
                        nc.gpsimd.dma_start(out=k_sb[:, 0:12, :],
                                            in_=k_d[t, :, 0:12, :])
                        nc.gpsimd.dma_start(out=v_sb[:, 0:12, :],
                                            in_=v_d[t, :, 0:12, :])
                        nc.gpsimd.dma_start(out=q_sb[:, 8:, :],
                                            in_=q_d[t, :, 8:, :])
                        nc.gpsimd.dma_start(out=k_sb[:, 12:, :],
                                            in_=k_d[t, :, 12:, :])
                        nc.gpsimd.dma_start(out=v_sb[:, 12:, :],
                                            in_=v_d[t, :, 12:, :])
                    else:
                        nc.gpsimd.dma_start(out=q_sb[:], in_=q_d[t])
                        nc.gpsimd.dma_start(out=k_sb[:], in_=k_d[t])
                        nc.gpsimd.dma_start(out=v_sb[:], in_=v_d[t])
                    qs.append(q_sb), ks.append(k_sb), vs.append(v_sb)
                return qs, ks, vs, deferred, heads

            def body_pipelined():
                """Software-pipelined emission: per slot i the engines see
                  DVE:  QK-TT(i) r0,r1   then AV-TT(i-1) r0,r1
                  PE :  QK-mm(i) x6      then OD-mm(i-1) x9
                  ACT:  exp(i) r0,r1
                  Pool: 1/D(i-2), out-mult(i-2)
                so every engine has ready work each slot. att is double-
                buffered per ROW ([128,9,128] f32, bank-aligned 512B/tap)
                which fits 2 bufs + od 2 bufs in the 16KB PSUM budget."""
                qs, ks, vs, deferred, heads = prefetch(defer=True)
                # one deferred prefetch trigger per early stageF slot
                trigs = []
                for q_sb, k_sb, v_sb, t in deferred:
                    trigs.append((k_sb, k_d[t]))
                    trigs.append((v_sb, v_d[t]))
                    trigs.append((q_sb, q_d[t]))
                total = n_chunks * n_sub
                prods, cts, atts, ods, osbs = {}, {}, {}, {}, {}

                def stageA(i):  # DVE: QK products
                    t, sb = divmod(i, n_sub)
                    rb = 2 * sb
                    prod = prodp.tile([128, 2, KK, W], F16, tag="prod")
                    prods[i] = prod
                    use_head = i == 0 and heads
                    k_t = heads["kh"] if use_head else ks[t]
                    q_t = heads["qh"] if use_head else qs[t]
                    for rl in range(2):
                        kv = _view(k_t, (rb + rl) * WP,
                                   [[2 * WP, 3], [2, 3], [1, W]])
                        qv = _view(q_t, (rb + rl) * W,
                                   [[0, 3], [0, 3], [1, W]])
                        pv = _view(prod, rl * KK * W,
                                   [[3 * W, 3], [W, 3], [1, W]])
                        nc.vector.tensor_tensor(
                            pv, kv, qv, mybir.AluOpType.mult)

                def stageB(i):  # PE: logits matmuls (per row, 4+4+1 taps)
                    # per-row att [128, 9, 128] f32: 512B/tap slot so every
                    # 448B matmul write stays inside a 2KB PSUM bank; row
                    # tiles ping-pong through the pool's 2 bufs
                    prod = prods[i]
                    row_atts = []
                    for rl in range(2):
                        att = att_ps.tile([128, KK, 128], F32, tag="att")
                        row_atts.append(att)
                        for g, ng in ((0, 4), (4, 4), (8, 1)):
                            rhs = _view(prod, rl * KK * W + g * W,
                                        [[W, ng], [1, W]])
                            out = _view(att, g * 128, [[128, ng], [1, W]])
                            nc.tensor.matmul(out, bs_sb[:], rhs,
                                             start=True, stop=True)
                    atts[i] = row_atts

                def stageC(i):  # ACT: exp per row -> c_t exp half
                    c_t = cp.tile([128, KK, 2, 224], F16, tag="C")
                    cts[i] = c_t
                    for rl in range(2):
                        att = atts[i][rl]
                        attn_v = _view(att, 0, [[128, KK], [1, W]])
                        exp_out = _view(c_t, 224 + rl * W,
                                        [[448, KK], [1, W]])
                        nc.scalar.activation(
                            exp_out, attn_v,
                            mybir.ActivationFunctionType.Exp)
                    del atts[i]

                def stageD(i):  # DVE: AV products
                    t, sb = divmod(i, n_sub)
                    rb = 2 * sb
                    c_t = cts[i]
                    v_t = heads["vh"] if (i == 0 and heads) else vs[t]
                    for rl in range(2):
                        vv = _view(v_t, (rb + rl) * WP,
                                   [[2 * WP, 3], [2, 3], [1, W]])
                        ev = _view(c_t, 224 + rl * W,
                                   [[3 * 448, 3], [448, 3], [1, W]])
                        p2 = _view(c_t, rl * W,
                                   [[3 * 448, 3], [448, 3], [1, W]])
                        nc.vector.tensor_tensor(
                            p2, vv, ev, mybir.AluOpType.mult)

                def stageE(i):  # PE: accumulate sum_k [prod | exp]
                    c_t = cts[i]
                    od = od_ps.tile([128, 448], F32, tag="od")
                    for kk in range(KK):
                        rhs = _view(c_t, kk * 448, [[1, 448]])
                        nc.tensor.matmul(od[:], id_sb[:], rhs,
                                         start=(kk == 0), stop=(kk == KK - 1))
                    del cts[i]
                    ods[i] = od

                def stageF(i):  # Pool: dr = 1/D ; o = Oun*dr ; out DMA
                    t, sb = divmod(i, n_sub)
                    rb = 2 * sb
                    if sb == 0:
                        osbs[t] = opool.tile([128, chunk_rows, W], F32,
                                             tag="o", name="o_sb")
                    o_sb = osbs[t]
                    od = ods.pop(i)
                    dr = drp.tile([128, 224], F32, tag="dr")
                    out_v = _view(o_sb, rb * W, [[1, 224]])
                    if norm_engine == "pool_div":
                        # Division exists on no engine; GPSIMD cannot touch
                        # PSUM. Spread the normalize across the three
                        # non-critical paths: DVE takes 1/D (only DVE has
                        # reciprocal), ACT copies the AV half out of PSUM,
                        # Pool does the final elementwise multiply in SBUF.
                        av_sb = drp.tile([128, 224], F32, tag="av_sb")
                        nc.scalar.copy(av_sb[:], od[:, 0:224])
                        if i % 4 == 3 and recip_act:
                            lnd = drp.tile([128, 224], F32, tag="lnd")
                            nc.scalar.activation(
                                lnd[:], od[:, 224:448],
                                mybir.ActivationFunctionType.Ln)
                            nc.scalar.activation(
                                dr[:], lnd[:],
                                mybir.ActivationFunctionType.Exp,
                                scale=-1.0)
                        else:
                            nc.vector.reciprocal(dr[:], od[:, 224:448])
                        nc.gpsimd.tensor_tensor(
                            out_v, av_sb[:], dr[:],
                            mybir.AluOpType.mult)
                    elif norm_engine == "dve":
                        nc.vector.reciprocal(dr[:], od[:, 224:448])
                        nc.vector.tensor_tensor(
                            out_v, od[:, 0:224], dr[:],
                            mybir.AluOpType.mult)
                    else:  # act
                        lnd = drp.tile([128, 224], F32, tag="lnd")
                        nc.scalar.activation(
                            lnd[:], od[:, 224:448],
                            mybir.ActivationFunctionType.Ln)
                        nc.scalar.activation(
                            dr[:], lnd[:],
                            mybir.ActivationFunctionType.Exp, scale=-1.0)
                        nc.vector.tensor_tensor(
                            out_v, od[:, 0:224], dr[:],
                            mybir.AluOpType.mult)
                    # output DMA: half at mid-chunk, then quarters so the
                    # tail transfer after the very last norm is small
                    if n_sub >= 8:
                        # DMA boundaries must sit on 2-row sub-block edges
                        h = 2 * (n_sub // 2)        # rows done by sb h/2-1
                        q3 = 2 * (3 * n_sub // 4)   # rows done by sb 3n/4-1
                        last = t == n_chunks - 1
                        if sb == n_sub // 2 - 1:
                            nc.sync.dma_start(out=o_d[t, :, 0:h, :],
                                              in_=o_sb[:, 0:h, :])
                        elif sb == 3 * n_sub // 4 - 1:
                            nc.sync.dma_start(out=o_d[t, :, h:q3, :],
                                              in_=o_sb[:, h:q3, :])
                        elif last and sb == n_sub - 2:
                            nc.sync.dma_start(
                                out=o_d[t, :, q3:chunk_rows - 2, :],
                                in_=o_sb[:, q3:chunk_rows - 2, :])
                        elif sb == n_sub - 1:
                            r0 = chunk_rows - 2 if last else q3
                            nc.sync.dma_start(out=o_d[t, :, r0:, :],
                                              in_=o_sb[:, r0:, :])
                    elif sb == n_sub - 1:
                        nc.sync.dma_start(out=o_d[t], in_=o_sb[:])

                for i in range(total + 2):
                    if i < total:
                        stageA(i)
                        stageB(i)
                        stageC(i)
                    if 0 <= i - 1 < total:
                        stageD(i - 1)
                        stageE(i - 1)
                    if 0 <= i - 2 < total:
                        stageF(i - 2)
                        if i - 2 >= 4 and (i - 2) % 2 == 0 and trigs:
                            sb_out, d_in = trigs.pop(0)
                            nc.gpsimd.dma_start(out=sb_out[:], in_=d_in)

            def body():
                # prefetch all chunks (cast f32 -> f16 during DMA).
                # Chunk 0 is loaded in two slices so the first sub-blocks
                # can start while the rest streams in.
                qs, ks, vs, _, _ = prefetch()

                for t in range(n_chunks):
                    q_sb, k_sb, v_sb = qs[t], ks[t], vs[t]
                    o_sb = opool.tile([128, chunk_rows, W], F32, tag="o")

                    for sb in range(n_sub):
                        rb = 2 * sb  # first out-row of sub-block
                        od = od_ps.tile([128, 448], F32, tag="od")
                        if True:
                            # prod: [128, 2(row), 9(kk), 112] fp16
                            prod = prodp.tile([128, 2, KK, W], F16,
                                              tag="prod")
                            # C: [128, 9(kk), 2(half), 224];
                            # half0=AV prod, half1=exp
                            c_t = cp.tile([128, KK, 2, 224], F16, tag="C")
                            attn_a = att_ps.tile([128, 5, 256], F32,
                                                 tag="attnA")
                            attn_b = att_ps.tile([128, 4, 256], F32,
                                                 tag="attnB")

                            # ---- QK products (per row, fused over 9 taps;
                            # walrus caps TT APs at 3 free dims).
                            # scalar_tensor_tensor (out = (in0*1)*in1) hits
                            # the DVE 4x perf mode; tensor_tensor caps at 2x.
                            if "qk_tt" not in skip:
                                for rl in range(2):
                                    kv = _view(k_sb, (rb + rl) * WP,
                                               [[2 * WP, 3], [2, 3], [1, W]])
                                    qv = _view(q_sb, (rb + rl) * W,
                                               [[0, 3], [0, 3], [1, W]])
                                    pv = _view(prod, rl * KK * W,
                                               [[3 * W, 3], [W, 3], [1, W]])
                                    if stt_products:
                                        nc.vector.scalar_tensor_tensor(
                                            pv, kv, 1.0, qv,
                                            mybir.AluOpType.mult,
                                            mybir.AluOpType.mult)
                                    else:
                                        nc.vector.tensor_tensor(
                                            pv, kv, qv, mybir.AluOpType.mult)

                            # ---- logits matmuls + exp per attn half
                            for att_t, k0, nk in ((attn_a, 0, 5),
                                                  (attn_b, 5, 4)):
                                if "qk_mm" not in skip:
                                    for kl in range(nk):
                                        kk = k0 + kl
                                        rhs = _view(prod, kk * W,
                                                    [[KK * W, 2], [1, W]])
                                        nc.tensor.matmul(
                                            att_t[:, kl, 0:224], bs_sb[:],
                                            rhs, start=True, stop=True)
                                if "exp" not in skip:
                                    attn_v = _view(att_t, 0,
                                                   [[256, nk], [1, 224]])
                                    exp_out = _view(c_t, k0 * 448 + 224,
                                                    [[448, nk], [1, 224]])
                                    nc.scalar.activation(
                                        exp_out, attn_v,
                                        mybir.ActivationFunctionType.Exp)

                            # ---- AV products (per row, fused over 9 taps)
                            if "av_tt" not in skip:
                                for rl in range(2):
                                    vv = _view(v_sb, (rb + rl) * WP,
                                               [[2 * WP, 3], [2, 3], [1, W]])
                                    ev = _view(c_t, 224 + rl * W,
                                               [[3 * 448, 3], [448, 3],
                                                [1, W]])
                                    p2 = _view(c_t, rl * W,
                                               [[3 * 448, 3], [448, 3],
                                                [1, W]])
                                    if stt_products:
                                        nc.vector.scalar_tensor_tensor(
                                            p2, vv, 1.0, ev,
                                            mybir.AluOpType.mult,
                                            mybir.AluOpType.mult)
                                    else:
                                        nc.vector.tensor_tensor(
                                            p2, vv, ev, mybir.AluOpType.mult)

                            # ---- accumulate sum_k [prod | exp] on PE
                            if "od_mm" not in skip:
                                for kk in range(KK):
                                    rhs = _view(c_t, kk * 448, [[1, 448]])
                                    nc.tensor.matmul(
                                        od[:], id_sb[:], rhs,
                                        start=(kk == 0), stop=(kk == KK - 1))

                        # ---- normalize: dr = 1/D; o = Oun*dr
                        if "norm" not in skip:
                            dr = drp.tile([128, 224], F32, tag="dr")
                            if dve_norm:
                                nc.vector.reciprocal(dr[:], od[:, 224:448])
                            else:
                                lnd = drp.tile([128, 224], F32, tag="lnd")
                                nc.scalar.activation(
                                    lnd[:], od[:, 224:448],
                                    mybir.ActivationFunctionType.Ln)
                                nc.scalar.activation(
                                    dr[:], lnd[:],
                                    mybir.ActivationFunctionType.Exp,
                                    scale=-1.0)
                            out_v = _view(o_sb, rb * W, [[1, 224]])
                            nc.vector.tensor_tensor(
                                out_v, od[:, 0:224], dr[:],
                                mybir.AluOpType.mult)

                        if n_sub >= 8 and sb == n_sub // 2 - 1:
                            nc.sync.dma_start(
                                out=o_d[t, :, 0:chunk_rows // 2, :],
                                in_=o_sb[:, 0:chunk_rows // 2, :])

                    if n_sub >= 8:
                        # second half only; first half was sent mid-chunk
                        nc.sync.dma_start(
                            out=o_d[t, :, chunk_rows // 2:, :],
                            in_=o_sb[:, chunk_rows // 2:, :])
                    else:
                        nc.sync.dma_start(out=o_d[t], in_=o_sb[:])

            # NB: tc.For_i emits raw-ISA register/branch ops this container's
            # walrus rejects ("ISA wrong length") -> python-unroll reps.
            for _ in range(reps):
                if pipeline:
                    body_pipelined()
                else:
                    body()

    return nc


def _split_waits(nc, max_waits=1):
    """walrus in this container rejects >1 sync-wait per instruction;
    move extra waits onto preceding NOPs."""
    for fn in nc.m.functions:
        for blk in fn.blocks:
            insts = blk.instructions
            new_insts = []
            for inst in insts:
                si = getattr(inst, "sync_info", None)
                if si is not None and si.on_wait and len(si.on_wait) > max_waits:
                    waits = list(si.on_wait)
                    extra, keep = waits[:-max_waits], waits[-max_waits:]
                    k = 0
                    while extra:
                        chunk, extra = extra[:max_waits], extra[max_waits:]
                        new_insts.append(mybir.InstNoOp(
                            name=f"{inst.name}-ws{k}",
                            engine=inst.engine,
                            sync_info=mybir.SyncInfo(on_wait=chunk,
                                                     on_update=[]),
                            bass_nofuse=True,
                        ))
                        k += 1
                    inst.sync_info = mybir.SyncInfo(
                        on_wait=keep, on_update=list(si.on_update))
                new_insts.append(inst)
            blk.instructions.clear()
            blk.instructions.extend(new_insts)


# ------------------------------------------------------------- host helpers
def make_consts():
    bs = np.zeros((128, 128), np.float16)
    for u in range(4):
        bs[u * 32:(u + 1) * 32, u * 32:(u + 1) * 32] = np.float16(SCALE)
    return bs, np.eye(128, dtype=np.float16)


def shard_inputs(q, k, v):
    """Full [4,192,112,112] f32 -> per-core input maps."""
    qf = np.ascontiguousarray(q.reshape(G, H, W))
    kp = np.pad(k, ((0, 0), (0, 0), (PAD, PAD), (PAD, PAD))).reshape(G, HP, WP)
    vp = np.pad(v, ((0, 0), (0, 0), (PAD, PAD), (PAD, PAD))).reshape(G, HP, WP)
    bs, id128 = make_consts()
    in_maps = []
    for c in range(N_CORES):
        qs, ks, vs = [], [], []
        for t in range(N_CHUNKS_PER_CORE):
            ci = c * N_CHUNKS_PER_CORE + t
            g, qt = divmod(ci, 4)
            r0 = CHUNK_ROWS * qt
            sl = slice(128 * g, 128 * (g + 1))
            qs.append(qf[sl, r0:r0 + CHUNK_ROWS, :])
            ks.append(kp[sl, r0:r0 + CHUNK_ROWS + 4, :])
            vs.append(vp[sl, r0:r0 + CHUNK_ROWS + 4, :])
        in_maps.append({
            "q": np.ascontiguousarray(np.stack(qs)),
            "k": np.ascontiguousarray(np.stack(ks)),
            "v": np.ascontiguousarray(np.stack(vs)),
            "bs": bs,
            "id128": id128,
        })
    return in_maps


def assemble_output(results):
    """Per-core 'o' [3,128,28,112] f32 -> full [4,112,112,192]."""
    oc = np.empty((G, H, W), np.float32)
    for c in range(N_CORES):
        for t in range(N_CHUNKS_PER_CORE):
            ci = c * N_CHUNKS_PER_CORE + t
            g, qt = divmod(ci, 4)
            r0 = CHUNK_ROWS * qt
            oc[128 * g:128 * (g + 1), r0:r0 + CHUNK_ROWS, :] = \
                results[c]["o"][t]
    return np.ascontiguousarray(
        oc.reshape(B, C, H, W).transpose(0, 2, 3, 1))


_NC_CACHE = {}


def kernel(q, k, v):
    key = "main"
    if key not in _NC_CACHE:
        nc_new = build_nc()
        _split_waits(nc_new)
        _NC_CACHE[key] = nc_new
    nc = _NC_CACHE[key]
    in_maps = shard_inputs(np.asarray(q), np.asarray(k), np.asarray(v))
    res = run_bass_kernel_spmd(nc, in_maps, list(range(N_CORES)))
    return assemble_output(res.results)

